# revision 2
# baseline (speedup 1.0000x reference)
"""Trainium2 Bass kernel for nn_LlamaAttention_45749991637119.

Mathematical structure of the reference: K/V are a single shared head that
is broadcast across all 64 query heads, and attention is computed per token
position (no cross-token mixing).  scores[b,t,h,g] = q[b,t,h]·k[b,t] is
independent of g, so the softmax over g is exactly uniform (1/64) and
attn[b,t,h,:] == v[b,t,:] for every head h.  Therefore

    out = (hidden @ Wv.T) @ Wo_sum.T,   Wo_sum[i,d] = sum_h Wo[i, 64h+d]

and Wq/Wk/cos/sin never influence the output (verified to 5e-7 rel err
against the reference).

Device schedule per core (1024 tokens, 4 pipelined token-groups of 256):

  stage A (v = Wv @ h^T): col-tiled 2x — even k-chunks accumulate into
    PSUM partitions 0-63 (PE array cols 0-63), odd chunks into partitions
    64-127, CONCURRENTLY (separate moving streams through separate col
    groups).  Produces a stacked [128, 256] psum: [vE; vO].
  stage B (out = v @ WoSum^T): the stacked vT (cast to bf16) is used as a
    K=128 stationary; the moving operand is WoSum^T REPLICATED on both
    partition halves, so the matmul itself computes vE·woS + vO·woS =
    v·woS — full-array K=128 matmuls, no partition-crossing add needed.
  drain: stage-B PSUM tiles are copied to SBUF split across BOTH the
    Vector and Scalar engines (row-block 0 -> DVE, row-block 1 -> ACT),
    since a single engine's PSUM->SBUF rate (~1 elem/lane/cycle) is the
    serial bottleneck otherwise.
  stores: one 1MB DMA per 128-token row-block on gpsimd (SWDGE), so
    output DMA overlaps later groups' input DMA / compute.

Sharding: data-parallel over tokens (B*T = 8192 -> 1024 per core).  All
inputs are packed on the host into ONE [128, 38912] bf16 tensor
(Wv^T chunks | WoSum^T x2 | hidden^T group-major) so input loads are a
few large contiguous DMAs with a simple semaphore protocol.
"""

import numpy as np

import concourse.bass as bass
import concourse.mybir as mybir
from concourse.bass_utils import run_bass_kernel_spmd

N_CORES = 8
B, T, HID = 4, 2048, 4096
D = 64                      # v dim (head_dim)
TOKS = (B * T) // N_CORES   # 1024 tokens per core
P = 128                     # partitions
KC = HID // P               # 32 k-chunks
TG = 256                    # token group
NG = TOKS // TG             # 4 groups
CD = 512                    # stage-B out-column tile
NCT = HID // CD             # 8 col tiles
NB = 6                      # stage-B psum ring
RB = TOKS // P              # 8 row-blocks (2 per group)

# packed input column offsets (bf16 elements per partition)
WV_COLS = KC * D            # 2048
WOS_COLS = HID              # 4096
HT_G_COLS = KC * TG         # 8192 per group
HT0 = WV_COLS + WOS_COLS    # 6144
PACK_COLS = HT0 + NG * HT_G_COLS  # 38912

COMPUTE_DTYPE = "bf16"
_CACHE = {}
LAST_RESULT = None


def _build():
    dt_in = mybir.dt.bfloat16

    nc = bass.Bass()
    pack = nc.dram_tensor("pack", [P, PACK_COLS], dt_in, kind="ExternalInput")
    out = nc.dram_tensor("out", [TOKS, HID], dt_in, kind="ExternalOutput")

    # ---- precomputed semaphore tick tables --------------------------------
    # PE emission order: A0, A1, B0, A2, B1, A3, B2, B3
    a_tick = {}
    b_tick = {}  # (g, i) -> pe tick, i = rb*8 + ct
    pe = 0
    emit_order = []
    for item in ["A0", "A1", "B0", "A2", "B1", "A3", "B2", "B3"]:
        g = int(item[1])
        if item[0] == "A":
            pe += 1
            a_tick[g] = pe
            emit_order.append(("A", g))
        else:
            for i in range(16):
                pe += 1
                b_tick[(g, i)] = pe
            emit_order.append(("B", g))

    # DVE: vt copies + row-block-0 drains; ACT: row-block-1 drains.
    vt_tick = {}
    drain_tick = {}  # (g, rb, ct) -> tick on its engine
    drain_on_dve = {}  # (g, rb, ct) -> bool
    dve = 0
    dve_prog = []  # ("vt", g) | ("dr", g, rb, ct)
    # DVE order must be monotone in the pe tick waited on:
    # vt0(1), vt1(2), dr B0 rb0 (3..10), vt2(19), dr B1 rb0, vt3(36), ...
    dve_order = [("vt", 0), ("vt", 1)]
    for g in range(NG):
        for ct in range(NCT):
            dve_order.append(("dr", g, 0, ct))
        if g + 2 < NG:
            dve_order.append(("vt", g + 2))
    for op in dve_order:
        dve += 1
        if op[0] == "vt":
            vt_tick[op[1]] = dve
        else:
            drain_tick[(op[1], op[2], op[3])] = dve
            drain_on_dve[(op[1], op[2], op[3])] = True
    dve_prog = dve_order

    act = 0
    act_prog = []
    for g in range(NG):
        for ct in range(NCT):
            act += 1
            drain_tick[(g, 1, ct)] = act
            drain_on_dve[(g, 1, ct)] = False
            act_prog.append(("dr", g, 1, ct))

    # s_load thresholds: pieces wv(16), ht g0(32), woS2(48), ht g1(64),
    # ht g2(80), ht g3(96)
    ht_thresh = {0: 32, 1: 64, 2: 80, 3: 96}
    WOS_THRESH = 48

    with (
        nc.sbuf_tensor([P, PACK_COLS], dt_in) as mega,
        nc.sbuf_tensor([P, RB * HID], dt_in) as out_sb,
        nc.sbuf_tensor([P, TOKS], dt_in) as vT,
        nc.psum_tensor([P, TG]) as psv0,
        nc.psum_tensor([P, TG]) as psv1,
        nc.psum_tensor([P, NB * CD]) as psB,
        nc.semaphore() as s_load,
        nc.semaphore() as s_pe,
        nc.semaphore() as s_dve,
        nc.semaphore() as s_act,
        nc.semaphore() as s_store,
        nc.Block() as block,
    ):
        psv = [psv0, psv1]

        def wv_chunk(c):
            return mega[:, c * D:(c + 1) * D]

        def woS2(ct):
            return mega[:, WV_COLS + ct * CD:WV_COLS + (ct + 1) * CD]

        def ht(g, c):
            base = HT0 + g * HT_G_COLS + c * TG
            return mega[:, base:base + TG]

        @block.sync
        def _(sync):
            pieces = [
                (0, WV_COLS),                                    # wv
                (HT0, HT0 + HT_G_COLS),                          # ht g0
                (WV_COLS, HT0),                                  # woS2
                (HT0 + HT_G_COLS, HT0 + 2 * HT_G_COLS),          # ht g1
                (HT0 + 2 * HT_G_COLS, HT0 + 3 * HT_G_COLS),      # ht g2
                (HT0 + 3 * HT_G_COLS, HT0 + 4 * HT_G_COLS),      # ht g3
            ]
            for lo, hi in pieces:
                sync.dma_start(out=mega[:, lo:hi], in_=pack[:, lo:hi]).then_inc(
                    s_load, 16
                )

        @block.tensor
        def _(tensor):
            waited = {}  # sem name -> max threshold emitted

            def wait(sem, name, val):
                if waited.get(name, 0) < val:
                    waited[name] = val
                    tensor.wait_ge(sem, val)

            for kind, g in emit_order:
                if kind == "A":
                    wait(s_load, "load", ht_thresh[g])
                    if g >= 2:
                        wait(s_dve, "dve", vt_tick[g - 2])
                    for c in range(KC):
                        half = c % 2
                        mm = tensor.matmul(
                            psv[g % 2][half * D:(half + 1) * D, :],
                            wv_chunk(c),
                            ht(g, c),
                            start=(c < 2),
                            stop=(c >= KC - 2),
                        )
                        if c == KC - 1:
                            mm.then_inc(s_pe, 1)
                else:
                    wait(s_dve, "dve", vt_tick[g])
                    if g == 0:
                        wait(s_load, "load", WOS_THRESH)
                    for i in range(16):
                        j = g * 16 + i
                        if j >= NB:
                            gp, ip = divmod(j - NB, 16)
                            key = (gp, ip // 8, ip % 8)
                            if drain_on_dve[key]:
                                wait(s_dve, "dve", drain_tick[key])
                            else:
                                wait(s_act, "act", drain_tick[key])
                        slot = j % NB
                        rb, ct = divmod(i, 8)
                        tensor.matmul(
                            psB[:, slot * CD:(slot + 1) * CD],
                            vT[:, (g * 2 + rb) * P:(g * 2 + rb + 1) * P],
                            woS2(ct),
                            start=True, stop=True,
                        ).then_inc(s_pe, 1)

        @block.vector
        def _(vector):
            for op in dve_prog:
                if op[0] == "vt":
                    g = op[1]
                    vector.wait_ge(s_pe, a_tick[g])
                    vector.tensor_copy(
                        out=vT[:, g * TG:(g + 1) * TG], in_=psv[g % 2][:, :]
                    ).then_inc(s_dve, 1)
                else:
                    _, g, rb, ct = op
                    i = rb * 8 + ct
                    j = g * 16 + i
                    vector.wait_ge(s_pe, b_tick[(g, i)])
                    slot = j % NB
                    r = g * 2 + rb
                    vector.tensor_copy(
                        out=out_sb[:, r * HID + ct * CD:r * HID + (ct + 1) * CD],
                        in_=psB[:, slot * CD:(slot + 1) * CD],
                    ).then_inc(s_dve, 1)

        @block.scalar
        def _(scalar):
            for _, g, rb, ct in act_prog:
                i = rb * 8 + ct
                j = g * 16 + i
                scalar.wait_ge(s_pe, b_tick[(g, i)])
                slot = j % NB
                r = g * 2 + rb
                scalar.activation(
                    out=out_sb[:, r * HID + ct * CD:r * HID + (ct + 1) * CD],
                    in_=psB[:, slot * CD:(slot + 1) * CD],
                    func=mybir.ActivationFunctionType.Copy,
                ).then_inc(s_act, 1)

        @block.gpsimd
        def _(gpsimd):
            for r in range(RB):
                g, rb = divmod(r, 2)
                key = (g, rb, NCT - 1)
                if drain_on_dve[key]:
                    gpsimd.wait_ge(s_dve, drain_tick[key])
                else:
                    gpsimd.wait_ge(s_act, drain_tick[key])
                gpsimd.dma_start(
                    out=out[r * P:(r + 1) * P, :],
                    in_=out_sb[:, r * HID:(r + 1) * HID],
                ).then_inc(s_store, 16)
            gpsimd.wait_ge(s_store, 16 * RB)
    return nc


def kernel(hidden_states, cos, sin, Wq, Wk, Wv, Wo):
    global LAST_RESULT
    import ml_dtypes
    np_bf16 = ml_dtypes.bfloat16

    if "nc" not in _CACHE:
        _CACHE["nc"] = _build()
    nc = _CACHE["nc"]

    hidden_states = np.asarray(hidden_states, dtype=np.float32)
    Wv = np.asarray(Wv, dtype=np.float32)
    Wo = np.asarray(Wo, dtype=np.float32)

    flat = hidden_states.reshape(B * T, HID)
    # Wv^T chunks: pack[p, c*64+d] = Wv[d, c*128+p]
    wv_part = np.ascontiguousarray(
        Wv.reshape(D, KC, P).transpose(2, 1, 0).reshape(P, KC * D)
    ).astype(np_bf16)
    # Wo_sum^T replicated on both partition halves: pack[p, j] = woS[p%64, j]
    woS = Wo.reshape(HID, HID // D, D).sum(axis=1, dtype=np.float32).T  # [64, 4096]
    woS2_part = np.concatenate([woS, woS], axis=0).astype(np_bf16)      # [128, 4096]

    in_maps = []
    for jc in range(N_CORES):
        blk = flat[jc * TOKS:(jc + 1) * TOKS, :]          # [1024, 4096]
        # ht group-major: pack[p, g*8192 + c*256 + t] = blk[g*256+t, c*128+p]
        ht_part = np.ascontiguousarray(
            blk.reshape(NG, TG, KC, P).transpose(3, 0, 2, 1).reshape(P, NG * HT_G_COLS)
        ).astype(np_bf16)
        packed = np.concatenate([wv_part, woS2_part, ht_part], axis=1)
        in_maps.append({"pack": np.ascontiguousarray(packed)})

    LAST_RESULT = run_bass_kernel_spmd(nc, in_maps, core_ids=list(range(N_CORES)))
    outs = [np.asarray(LAST_RESULT.results[jc]["out"]).astype(np.float32)
            for jc in range(N_CORES)]
    return np.concatenate(outs, axis=0).reshape(B, T, HID)


# revision 4
# speedup vs baseline: 1.2142x; 1.2142x over previous
"""Trainium2 Bass kernel for nn_LlamaAttention_45749991637119.

Mathematical structure of the reference: K/V are a single shared head that
is broadcast across all 64 query heads, and attention is computed per token
position (no cross-token mixing).  scores[b,t,h,g] = q[b,t,h]·k[b,t] is
independent of g, so the softmax over g is exactly uniform (1/64) and
attn[b,t,h,:] == v[b,t,:] for every head h.  Therefore

    out = (hidden @ Wv.T) @ Wo_sum.T,   Wo_sum[i,d] = sum_h Wo[i, 64h+d]

and Wq/Wk/cos/sin never influence the output (verified to 5e-7 rel err
against the reference).

Device schedule per core (1024 tokens, 4 pipelined token-groups of 256):

  stage A (v = Wv @ h^T): col-tiled 2x — even k-chunks accumulate into
    PSUM partitions 0-63 (PE array cols 0-63), odd chunks into partitions
    64-127, CONCURRENTLY (separate moving streams through separate col
    groups).  Produces a stacked [128, 256] psum: [vE; vO].
  stage B (out = v @ WoSum^T): the stacked vT (cast to bf16) is used as a
    K=128 stationary; the moving operand is WoSum^T REPLICATED on both
    partition halves, so the matmul itself computes vE·woS + vO·woS =
    v·woS — full-array K=128 matmuls, no partition-crossing add needed.
  drain: stage-B PSUM tiles are copied to SBUF in 1024-col PAIRS split
    across BOTH the Vector and Scalar engines (row-block 0 -> DVE,
    row-block 1 -> ACT; vT copies on ACT), since a single engine's
    PSUM->SBUF rate (~1 elem/lane/cycle) would be the serial bottleneck.
  stores: one 1MB DMA per 128-token row-block on gpsimd (SWDGE), so
    output DMA overlaps later groups' input DMA / compute.

PE program order A0 A1 B0 A2 B1 B2 A3 B3 keeps the tail short: B2 (data
ready early) runs while the last input piece is still in flight, and only
A3+B3 remain after the input DMA completes.  ht loads are split in half
so each stage-A group starts as soon as its first 16 k-chunks land.

Sharding: data-parallel over tokens (B*T = 8192 -> 1024 per core).  All
inputs are packed on the host into ONE [128, 38912] bf16 tensor
(Wv^T chunks | WoSum^T x2 | hidden^T group-major).
"""

import numpy as np

import concourse.bass as bass
import concourse.mybir as mybir
from concourse.bass_utils import run_bass_kernel_spmd

N_CORES = 8
B, T, HID = 4, 2048, 4096
D = 64                      # v dim (head_dim)
TOKS = (B * T) // N_CORES   # 1024 tokens per core
P = 128                     # partitions
KC = HID // P               # 32 k-chunks
TG = 256                    # token group
NG = TOKS // TG             # 4 groups
CD = 512                    # stage-B out-column tile
NCT = HID // CD             # 8 col tiles
NB = 6                      # stage-B psum ring (3 drain-pairs)
RB = TOKS // P              # 8 row-blocks (2 per group)

# packed input column offsets (bf16 elements per partition)
WV_COLS = KC * D            # 2048
WOS_COLS = HID              # 4096
HT_G_COLS = KC * TG         # 8192 per group
HT0 = WV_COLS + WOS_COLS    # 6144
PACK_COLS = HT0 + NG * HT_G_COLS  # 38912

COMPUTE_DTYPE = "bf16"
_CACHE = {}
LAST_RESULT = None

# PE emission order (A groups + B groups)
PE_ORDER = [("A", 0), ("A", 1), ("B", 0), ("A", 2), ("B", 1), ("B", 2),
            ("A", 3), ("B", 3)]


def _ticks():
    """Precompute semaphore tick tables for all engines."""
    a_tick, b_tick = {}, {}
    pe = 0
    for kind, g in PE_ORDER:
        if kind == "A":
            pe += 1
            a_tick[g] = pe
        else:
            for i in range(16):
                pe += 1
                b_tick[(g, i)] = pe

    # DVE: rb0 drain-pairs only.  ACT: vT copies + rb1 drain-pairs.
    # Each program must be monotone in the pe tick it waits on.
    dve_prog, act_prog = [], []
    for kind, g in PE_ORDER:
        if kind == "A":
            act_prog.append(("vt", g))
        else:
            for pi in range(4):
                dve_prog.append(("dr", g, 0, pi))
                act_prog.append(("dr", g, 1, pi))

    vt_tick, pair_tick, pair_on_dve = {}, {}, {}
    t = 0
    for op in dve_prog:
        t += 1
        pair_tick[op[1:]] = t
        pair_on_dve[op[1:]] = True
    t = 0
    for op in act_prog:
        t += 1
        if op[0] == "vt":
            vt_tick[op[1]] = t
        else:
            pair_tick[op[1:]] = t
            pair_on_dve[op[1:]] = False
    return a_tick, b_tick, dve_prog, act_prog, vt_tick, pair_tick, pair_on_dve


def _build():
    dt_in = mybir.dt.bfloat16

    nc = bass.Bass()
    pack = nc.dram_tensor("pack", [P, PACK_COLS], dt_in, kind="ExternalInput")
    out = nc.dram_tensor("out", [TOKS, HID], dt_in, kind="ExternalOutput")

    (a_tick, b_tick, dve_prog, act_prog, vt_tick, pair_tick,
     pair_on_dve) = _ticks()

    # s_load thresholds: wv, g0a, g0b, woS2, g1a, g1b, g2a, g2b, g3a, g3b
    ht_a = {0: 32, 1: 80, 2: 112, 3: 144}
    ht_b = {0: 48, 1: 96, 2: 128, 3: 160}
    WOS_THRESH = 64

    with (
        nc.sbuf_tensor([P, PACK_COLS], dt_in) as mega,
        nc.sbuf_tensor([P, RB * HID], dt_in) as out_sb,
        nc.sbuf_tensor([P, TOKS], dt_in) as vT,
        nc.psum_tensor([P, TG]) as psv0,
        nc.psum_tensor([P, TG]) as psv1,
        nc.psum_tensor([P, NB * CD]) as psB,
        nc.semaphore() as s_load,
        nc.semaphore() as s_pe,
        nc.semaphore() as s_dve,
        nc.semaphore() as s_act,
        nc.semaphore() as s_store,
        nc.Block() as block,
    ):
        psv = [psv0, psv1]

        def wv_chunk(c):
            return mega[:, c * D:(c + 1) * D]

        def woS2(ct):
            return mega[:, WV_COLS + ct * CD:WV_COLS + (ct + 1) * CD]

        def ht(g, c):
            base = HT0 + g * HT_G_COLS + c * TG
            return mega[:, base:base + TG]

        def out_sb_cols(r, c0, c1):
            return out_sb[:, r * HID + c0:r * HID + c1]

        @block.sync
        def _(sync):
            half = HT_G_COLS // 2
            pieces = [(0, WV_COLS),
                      (HT0, HT0 + half), (HT0 + half, HT0 + HT_G_COLS),
                      (WV_COLS, HT0)]
            for g in range(1, NG):
                lo = HT0 + g * HT_G_COLS
                pieces += [(lo, lo + half), (lo + half, lo + HT_G_COLS)]
            for lo, hi in pieces:
                sync.dma_start(out=mega[:, lo:hi], in_=pack[:, lo:hi]).then_inc(
                    s_load, 16
                )

        @block.tensor
        def _(tensor):
            waited = {}

            def wait(sem, name, val):
                if waited.get(name, 0) < val:
                    waited[name] = val
                    tensor.wait_ge(sem, val)

            for kind, g in PE_ORDER:
                if kind == "A":
                    if g >= 2:
                        wait(s_act, "act", vt_tick[g - 2])
                    for c in range(KC):
                        if c == 0:
                            wait(s_load, "load", ht_a[g])
                        elif c == KC // 2:
                            wait(s_load, "load", ht_b[g])
                        half = c % 2
                        mm = tensor.matmul(
                            psv[g % 2][half * D:(half + 1) * D, :],
                            wv_chunk(c),
                            ht(g, c),
                            start=(c < 2),
                            stop=(c >= KC - 2),
                        )
                        if c == KC - 1:
                            mm.then_inc(s_pe, 1)
                else:
                    wait(s_act, "act", vt_tick[g])
                    if g == 0:
                        wait(s_load, "load", WOS_THRESH)
                    for i in range(16):
                        j = g * 16 + i
                        if j >= NB:
                            gp, ip = divmod(j - NB, 16)
                            key = (gp, ip // 8, (ip % 8) // 2)
                            if pair_on_dve[key]:
                                wait(s_dve, "dve", pair_tick[key])
                            else:
                                wait(s_act, "act", pair_tick[key])
                        slot = j % NB
                        rb, ct = divmod(i, 8)
                        tensor.matmul(
                            psB[:, slot * CD:(slot + 1) * CD],
                            vT[:, (g * 2 + rb) * P:(g * 2 + rb + 1) * P],
                            woS2(ct),
                            start=True, stop=True,
                        ).then_inc(s_pe, 1)

        @block.vector
        def _(vector):
            for _, g, rb, pi in dve_prog:
                i = rb * 8 + 2 * pi
                j = g * 16 + i
                vector.wait_ge(s_pe, b_tick[(g, i + 1)])
                slot = j % NB
                r = g * 2 + rb
                vector.tensor_copy(
                    out=out_sb_cols(r, 2 * pi * CD, (2 * pi + 2) * CD),
                    in_=psB[:, slot * CD:(slot + 2) * CD],
                ).then_inc(s_dve, 1)

        @block.scalar
        def _(scalar):
            for op in act_prog:
                if op[0] == "vt":
                    g = op[1]
                    scalar.wait_ge(s_pe, a_tick[g])
                    scalar.activation(
                        out=vT[:, g * TG:(g + 1) * TG],
                        in_=psv[g % 2][:, :],
                        func=mybir.ActivationFunctionType.Copy,
                    ).then_inc(s_act, 1)
                else:
                    _, g, rb, pi = op
                    i = rb * 8 + 2 * pi
                    j = g * 16 + i
                    scalar.wait_ge(s_pe, b_tick[(g, i + 1)])
                    slot = j % NB
                    r = g * 2 + rb
                    scalar.activation(
                        out=out_sb_cols(r, 2 * pi * CD, (2 * pi + 2) * CD),
                        in_=psB[:, slot * CD:(slot + 2) * CD],
                        func=mybir.ActivationFunctionType.Copy,
                    ).then_inc(s_act, 1)

        @block.gpsimd
        def _(gpsimd):
            for r in range(RB):
                g, rb = divmod(r, 2)
                key = (g, rb, 3)
                if pair_on_dve[key]:
                    gpsimd.wait_ge(s_dve, pair_tick[key])
                else:
                    gpsimd.wait_ge(s_act, pair_tick[key])
                gpsimd.dma_start(
                    out=out[r * P:(r + 1) * P, :],
                    in_=out_sb[:, r * HID:(r + 1) * HID],
                ).then_inc(s_store, 16)
            gpsimd.wait_ge(s_store, 16 * RB)
    return nc


def kernel(hidden_states, cos, sin, Wq, Wk, Wv, Wo):
    global LAST_RESULT
    import ml_dtypes
    np_bf16 = ml_dtypes.bfloat16

    if "nc" not in _CACHE:
        _CACHE["nc"] = _build()
    nc = _CACHE["nc"]

    hidden_states = np.asarray(hidden_states, dtype=np.float32)
    Wv = np.asarray(Wv, dtype=np.float32)
    Wo = np.asarray(Wo, dtype=np.float32)

    flat = hidden_states.reshape(B * T, HID)
    # Wv^T chunks: pack[p, c*64+d] = Wv[d, c*128+p]
    wv_part = np.ascontiguousarray(
        Wv.reshape(D, KC, P).transpose(2, 1, 0).reshape(P, KC * D)
    ).astype(np_bf16)
    # Wo_sum^T replicated on both partition halves: pack[p, j] = woS[p%64, j]
    woS = Wo.reshape(HID, HID // D, D).sum(axis=1, dtype=np.float32).T  # [64, 4096]
    woS2_part = np.concatenate([woS, woS], axis=0).astype(np_bf16)      # [128, 4096]

    in_maps = []
    for jc in range(N_CORES):
        blk = flat[jc * TOKS:(jc + 1) * TOKS, :]          # [1024, 4096]
        # ht group-major: pack[p, g*8192 + c*256 + t] = blk[g*256+t, c*128+p]
        ht_part = np.ascontiguousarray(
            blk.reshape(NG, TG, KC, P).transpose(3, 0, 2, 1).reshape(P, NG * HT_G_COLS)
        ).astype(np_bf16)
        packed = np.concatenate([wv_part, woS2_part, ht_part], axis=1)
        in_maps.append({"pack": np.ascontiguousarray(packed)})

    LAST_RESULT = run_bass_kernel_spmd(nc, in_maps, core_ids=list(range(N_CORES)))
    outs = [np.asarray(LAST_RESULT.results[jc]["out"]).astype(np.float32)
            for jc in range(N_CORES)]
    return np.concatenate(outs, axis=0).reshape(B, T, HID)
